# revision 16
# baseline (speedup 1.0000x reference)
"""Trainium2 Bass kernel for nn_EquivariantScalar (segment_reduce).

Strategy (8 NeuronCores, atom-dim sharding):
  - 200000 atoms split 25000/core, zero-padded to 25088 = 49 tiles x 512.
  - Activations kept feature-major in SBUF ([feature, atom]); host pre-
    transposes s, v and the batch mask so every DMA is wide/contiguous.
  - All heavy matmuls run as float32r (full fp32 storage, PE streams
    1 col/cycle like bf16, ~1.5e-4 rel err vs 4x-slower plain fp32).
  - Per-atom chain: v2 = Wv2 v; norm = sqrt(sum_c (v2+eps)^2);
    h1 = Wh1s s + Wh1n norm; g = silu(h1 + b1); sg/ss = Wsg g / Wss g;
    vnew = (Wv1 v) * (ss + bss) ... second block the same, except the
    unused second-block v-output (u1_w) and the ss half of a1_w2 are
    skipped, and the final s @ out_w.T + out_b is folded into a single
    128-vector weff / scalar beff applied to g2 of block 1.
  - Masked pooling: per 128-atom group z^T comes straight out of the PE
    (lhsT = g2 slice), then one fp16 matmul against the one-hot batch
    rows accumulates y[1,256] in a persistent PSUM bank. Host sums the
    8 per-core partials.
"""
import os
import sys
import types

import numpy as np

NA = 200000
B = 256
NF = 128
NCORES = 8
SH = NA // NCORES          # 25000 atoms per core
T = 512                    # atoms per tile
NT = 49                    # tiles per core
NAs = NT * T               # 25088 padded atoms per core
EPS = 1e-8

_prog_cache = {}


def _install_compat_patches():
    """Patches for this container: split multi-wait instructions (this
    walrus caps non-EventSemaphore instructions at ONE sync wait) and
    register the axon NTFF profile hook if tracing is requested."""
    import bass_rust
    from concourse import tile
    from concourse.vector_clock import ScopedClock

    if getattr(tile.TileContext, "_wait_split_patched", False):
        return

    def _patched_drain_and_barrier(self, tick_clock, wait_clock):
        nc = self.nc
        drain_inst = nc.sync.drain()
        wait_clock.add_sem_waits(
            drain_inst.ins, ScopedClock({None: tick_clock.global_clock})
        )
        si = drain_inst.ins.sync_info
        if si is not None and len(si.on_wait) > 1:
            waits = list(si.on_wait)
            si.on_wait = waits[:1]
            for w in waits[1:]:
                n = nc.sync.nop(nofuse=True, hint="tail_drain_wait_split")
                n.ins.sync_info = bass_rust.SyncInfo(on_wait=[w], on_update=[])
        nc.all_engine_barrier()
        assert self.sems is not None
        popped = nc._tile_sem_poison_stack.pop()
        assert popped is self._sem_poison
        nc.clear_and_free_semaphores(list(self.sems.allocated().values()))
        nc.all_engine_barrier()

    tile.TileContext._drain_and_barrier = _patched_drain_and_barrier
    tile.TileContext._wait_split_patched = True


def _legalize_waits(nc):
    """Hoist extra sync waits (beyond the per-instruction HW capacity)
    onto fresh single-wait NoOps inserted just before, same engine."""
    import bass_rust
    import concourse.mybir as mybir

    counter = [0]
    for fn in nc.m.functions:
        for bb in fn.blocks:
            out = []
            changed = False
            for inst in bb.instructions:
                si = getattr(inst, "sync_info", None)
                waits = list(si.on_wait) if si is not None else []
                cap = 2 if isinstance(inst, mybir.InstEventSemaphore) else 1
                if len(waits) > cap:
                    si.on_wait = waits[:cap]
                    for w in waits[cap:]:
                        counter[0] += 1
                        n = mybir.InstNoOp(name=f"waitsplit-{counter[0]}")
                        n.engine = inst.engine
                        n.sync_info = bass_rust.SyncInfo(on_wait=[w], on_update=[])
                        out.append(n)
                    changed = True
                out.append(inst)
            if changed:
                bb.instructions = out


def _maybe_install_trace_shim():
    """Optional: NTFF profiling under axon needs antenv.axon_hooks."""
    try:
        import antenv

        if "antenv.axon_hooks" in sys.modules:
            return
        mod = types.ModuleType("antenv.axon_hooks")
        hook = [None]
        mod.set_axon_ntff_profile_hook = lambda h: hook.__setitem__(0, h)
        mod.get_axon_ntff_profile_hook = lambda: hook[0]
        sys.modules["antenv.axon_hooks"] = mod
        antenv.axon_hooks = mod
        from trn_agent_boot.trn_boot import _ntff_profile_via_ctypes

        mod.set_axon_ntff_profile_hook(
            _ntff_profile_via_ctypes("/opt/axon/libaxon_pjrt.so")
        )
    except Exception:
        pass


def _build_program():
    import concourse.bass as bass
    import concourse.mybir as mybir
    from concourse.tile import TileContext
    from concourse.tile_rust import add_dep_helper

    F = mybir.dt.float32
    FR = mybir.dt.float32r
    F16 = mybir.dt.float16
    AF = mybir.ActivationFunctionType

    nc = bass.Bass()

    vT = nc.dram_tensor("vT", [3, NF, NAs], FR, kind="ExternalInput")
    sT = nc.dram_tensor("sT", [NF, NAs], FR, kind="ExternalInput")
    bT = nc.dram_tensor("bT", [NAs, B], F16, kind="ExternalInput")

    wnames = [
        "wv1_0", "wv2_0", "wh1s_0", "wh1n_0", "wsg_0", "wss_0",
        "wv2_1", "wh1s_1", "wh1n_1",
    ]
    wdram = {n: nc.dram_tensor(n, [NF, NF], FR, kind="ExternalInput") for n in wnames}
    weff_d = nc.dram_tensor("weff", [NF, 1], F16, kind="ExternalInput")
    bias_names = ["b1_0", "bsg_0", "bss_0", "b1_1", "beff", "eps"]
    bdram = {n: nc.dram_tensor(n, [NF, 1], F, kind="ExternalInput") for n in bias_names}

    y = nc.dram_tensor("y", [1, B], F, kind="ExternalOutput")

    with TileContext(nc) as tc:
        with (
            tc.tile_pool(name="wp", bufs=1) as wp,
            tc.tile_pool(name="vin", bufs=3) as vin_p,
            tc.tile_pool(name="sin", bufs=3) as sin_p,
            tc.tile_pool(name="bin", bufs=3) as bin_p,
            tc.tile_pool(name="sq", bufs=2) as sq_p,
            tc.tile_pool(name="nrm", bufs=2) as nrm_p,
            tc.tile_pool(name="g2", bufs=2) as g2_p,
            tc.tile_pool(name="sgss", bufs=2) as sgss_p,
            tc.tile_pool(name="vn", bufs=2) as vn_p,
            tc.tile_pool(name="zz", bufs=2) as zz_p,
            tc.tile_pool(name="yo", bufs=1) as yo_p,
            tc.tile_pool(name="psA", bufs=1, space="PSUM") as psA,
            tc.tile_pool(name="psB", bufs=1, space="PSUM") as psB,
            tc.tile_pool(name="psH", bufs=2, space="PSUM") as psH,
        ):
            w = {}
            for n in wnames:
                w[n] = wp.tile([NF, NF], FR, tag=n, name=n)
                nc.sync.dma_start(out=w[n][:], in_=wdram[n][:])
            weff = wp.tile([NF, 1], F16, tag="weff")
            nc.sync.dma_start(out=weff[:], in_=weff_d[:])
            bias = {}
            for n in bias_names:
                bias[n] = wp.tile([NF, 1], F, tag=n, name=n)
                nc.sync.dma_start(out=bias[n][:], in_=bdram[n][:])

            _last_act = [None]

            def _act(*args, **kw):
                inst = nc.scalar.activation(*args, **kw)
                if _last_act[0] is not None:
                    add_dep_helper(
                        inst.ins, _last_act[0].ins, sync=False,
                        reason="pin ACT table-set phase order",
                    )
                _last_act[0] = inst
                return inst

            yacc = yo_p.tile([1, B], F, tag="yacc0", name="yacc")
            nc.gpsimd.memset(yacc[:], 0.0)

            # One-tile-skewed pipeline: iteration i runs block-0 of tile i
            # (through the v2-of-block-1 matmuls) and the block-1 tail of
            # tile i-1. ACT instruction stream per iteration is grouped
            # into a sqrt-set phase then a silu-set phase (Square/Identity
            # are fillers present in every set), so the activation table
            # reloads only twice per tile instead of four times.
            st_cur = None
            pv2b_prev = None
            g2b_prev = None
            sg_prev = None
            bt_prev = None

            for i in range(NT + 1):
                cur = i if i < NT else None
                prev = i - 1 if i > 0 else None

                if cur is not None:
                    a0 = cur * T
                    vt = vin_p.tile([NF, 3, T], FR, tag="vt", name="vt")
                    nc.sync.dma_start(
                        out=vt[:],
                        in_=vT[:, :, a0 : a0 + T].rearrange("c f a -> f c a"),
                    )
                    st_cur = sin_p.tile([NF, T], FR, tag="st", name="st")
                    nc.sync.dma_start(out=st_cur[:], in_=sT[:, a0 : a0 + T])
                    bt = bin_p.tile([128, 4, B], F16, tag="bt", name="bt")
                    nc.sync.dma_start(
                        out=bt[:],
                        in_=bT[a0 : a0 + T, :].rearrange("(g p) b -> p g b", p=128),
                    )

                    # block-0 v2 matmuls
                    pv2a = psA.tile([128, 3 * T], F, tag="bigA", name="pv2a")
                    for c in range(3):
                        nc.tensor.matmul(
                            pv2a[:, c * T : (c + 1) * T], w["wv2_0"][:], vt[:, c, :],
                            start=True, stop=True,
                        )
                    # h1 s-halves: inputs are already resident, run them early
                    ph1a = psH.tile([128, T], F, tag="h", name="ph1a")
                    nc.tensor.matmul(
                        ph1a[:], w["wh1s_0"][:], st_cur[:], start=True, stop=False,
                        skip_group_check=True,
                    )
                if prev is not None:
                    ph1b = psH.tile([128, T], F, tag="h", name="ph1b")
                    nc.tensor.matmul(
                        ph1b[:], w["wh1s_1"][:], sg_prev[:], start=True, stop=False,
                        skip_group_check=True,
                    )
                if cur is not None:
                    pass

                # ---- sqrt-set ACT phase (Square is a filler: no reload) ----
                if cur is not None:
                    sqa = sq_p.tile([128, 3 * T], F, tag="sqa", name="sqa")
                    _act(sqa[:], pv2a[:], AF.Square, bias=bias["eps"][:])
                if prev is not None:
                    sqb = sq_p.tile([128, 3 * T], F, tag="sqb", name="sqb")
                    _act(
                        sqb[:], pv2b_prev[:], AF.Square, bias=bias["eps"][:]
                    )
                if cur is not None:
                    n01a = nrm_p.tile([128, T], F, tag="n01a", name="n01a")
                    nc.vector.tensor_add(n01a[:], sqa[:, 0:T], sqa[:, T : 2 * T])
                    n2a = nrm_p.tile([128, T], F, tag="n2a", name="n2a")
                    nc.vector.tensor_add(n2a[:], n01a[:], sqa[:, 2 * T : 3 * T])
                    norm0 = nrm_p.tile([128, T], FR, tag="norm0", name="norm0")
                    _act(norm0[:], n2a[:], AF.Sqrt)
                if prev is not None:
                    n01b = nrm_p.tile([128, T], F, tag="n01b", name="n01b")
                    nc.vector.tensor_add(n01b[:], sqb[:, 0:T], sqb[:, T : 2 * T])
                    n2b = nrm_p.tile([128, T], F, tag="n2b", name="n2b")
                    nc.vector.tensor_add(n2b[:], n01b[:], sqb[:, 2 * T : 3 * T])
                    norm1 = nrm_p.tile([128, T], FR, tag="norm1", name="norm1")
                    _act(norm1[:], n2b[:], AF.Sqrt)

                # ---- h1 norm-half accumulations ----
                if cur is not None:
                    nc.tensor.matmul(
                        ph1a[:], w["wh1n_0"][:], norm0[:], start=False, stop=True,
                        skip_group_check=True,
                    )
                if prev is not None:
                    nc.tensor.matmul(
                        ph1b[:], w["wh1n_1"][:], norm1[:], start=False, stop=True,
                        skip_group_check=True,
                    )

                # ---- silu-set ACT phase ----
                if cur is not None:
                    g2a = g2_p.tile([128, T], FR, tag="g2a", name="g2a")
                    _act(g2a[:], ph1a[:], AF.Silu, bias=bias["b1_0"][:])
                if prev is not None:
                    g2b_prev = g2_p.tile([128, T], F16, tag="g2b", name="g2b")
                    _act(
                        g2b_prev[:], ph1b[:], AF.Silu, bias=bias["b1_1"][:]
                    )
                if cur is not None:
                    psg = psH.tile([128, T], F, tag="h", name="psg")
                    nc.tensor.matmul(psg[:], w["wsg_0"][:], g2a[:], start=True, stop=True)
                    sg_new = sgss_p.tile([128, T], FR, tag="sg", name="sg_new")
                    _act(
                        sg_new[:], psg[:], AF.Identity, bias=bias["bsg_0"][:]
                    )
                    pss = psH.tile([128, T], F, tag="h", name="pss")
                    nc.tensor.matmul(pss[:], w["wss_0"][:], g2a[:], start=True, stop=True)
                    ss = sgss_p.tile([128, T], F, tag="ss", name="ss")
                    nc.vector.tensor_scalar_add(ss[:], pss[:], bias["bss_0"][:])

                # ---- block-1 tail of prev: z + pooling ----
                if prev is not None:
                    pz = psH.tile([128, T], F, tag="h", name="pz")
                    for g in range(4):
                        nc.tensor.matmul(
                            pz[:, g : g + 1],
                            g2b_prev[:, g * 128 : (g + 1) * 128],
                            weff[:],
                            start=True, stop=True,
                        )
                    zt = zz_p.tile([128, 4], F16, tag="z", name="zt")
                    nc.vector.tensor_scalar_add(zt[:], pz[:, 0:4], bias["beff"][:])
                    ypart = psH.tile([1, B], F, tag="h", name="ypart")
                    for g in range(4):
                        nc.tensor.matmul(
                            ypart[:],
                            zt[:, g : g + 1],
                            bt_prev[:, g, :],
                            start=(g == 0),
                            stop=(g == 3),
                            skip_group_check=True,
                        )
                    yacc_new = yo_p.tile(
                        [1, B], F, tag=f"yacc{(prev + 1) % 2}", name="yacc_new"
                    )
                    nc.vector.tensor_add(yacc_new[:], yacc[:], ypart[:])
                    yacc = yacc_new

                # ---- block-0 v1 matmuls + gating muls, then block-1 v2 ----
                if cur is not None:
                    vnew = vn_p.tile([128, 3, T], FR, tag="vn", name="vnew")
                    for c in range(3):
                        pv1c = psH.tile([128, T], F, tag="h", name="pv1c")
                        nc.tensor.matmul(
                            pv1c[:], w["wv1_0"][:], vt[:, c, :], start=True, stop=True
                        )
                        nc.vector.tensor_mul(vnew[:, c, :], pv1c[:], ss[:])
                    pv2b_prev = psB.tile([128, 3 * T], F, tag="bigB", name="pv2b")
                    for c in range(3):
                        nc.tensor.matmul(
                            pv2b_prev[:, c * T : (c + 1) * T],
                            w["wv2_1"][:], vnew[:, c, :],
                            start=True, stop=True,
                        )
                    sg_prev = sg_new
                    bt_prev = bt

            nc.sync.dma_start(out=y[:], in_=yacc[:])

    _legalize_waits(nc)
    return nc


def kernel(**inputs):
    _install_compat_patches()
    if os.environ.get("BASS_TRACE"):
        _maybe_install_trace_shim()
    from concourse.bass_utils import run_bass_kernel_spmd

    s = np.asarray(inputs["s"], np.float32)
    v = np.asarray(inputs["v"], np.float32)
    batch = np.asarray(inputs["batch"], np.float32)

    f32 = lambda a: np.ascontiguousarray(np.asarray(a, np.float32))
    u0_w, v0_w = f32(inputs["u0_w"]), f32(inputs["v0_w"])
    a0_w1, a0_b1 = f32(inputs["a0_w1"]), f32(inputs["a0_b1"])
    a0_w2, a0_b2 = f32(inputs["a0_w2"]), f32(inputs["a0_b2"])
    v1_w = f32(inputs["v1_w"])
    a1_w1, a1_b1 = f32(inputs["a1_w1"]), f32(inputs["a1_b1"])
    a1_w2, a1_b2 = f32(inputs["a1_w2"]), f32(inputs["a1_b2"])
    out_w, out_b = f32(inputs["out_w"]), f32(inputs["out_b"])

    weights = {
        "wv1_0": np.ascontiguousarray(u0_w.T),
        "wv2_0": np.ascontiguousarray(v0_w.T),
        "wh1s_0": np.ascontiguousarray(a0_w1.T[:NF]),
        "wh1n_0": np.ascontiguousarray(a0_w1.T[NF:]),
        "wsg_0": np.ascontiguousarray(a0_w2[:NF].T),
        "wss_0": np.ascontiguousarray(a0_w2[NF:].T),
        "wv2_1": np.ascontiguousarray(v1_w.T),
        "wh1s_1": np.ascontiguousarray(a1_w1.T[:NF]),
        "wh1n_1": np.ascontiguousarray(a1_w1.T[NF:]),
        "weff": np.ascontiguousarray((out_w[0] @ a1_w2[:NF]).reshape(NF, 1)).astype(np.float16),
        "b1_0": np.ascontiguousarray(a0_b1.reshape(NF, 1)),
        "bsg_0": np.ascontiguousarray(a0_b2[:NF].reshape(NF, 1)),
        "bss_0": np.ascontiguousarray(a0_b2[NF:].reshape(NF, 1)),
        "b1_1": np.ascontiguousarray(a1_b1.reshape(NF, 1)),
        "beff": np.full(
            (NF, 1), float(out_w[0] @ a1_b2[:NF] + out_b[0]), np.float32
        ),
        "eps": np.full((NF, 1), EPS, np.float32),
    }

    v0 = v[0]            # (NA, 3, NF)
    s0 = s[0]            # (NA, NF)
    bm = batch[:, :, 0]  # (B, NA)

    in_maps = []
    for c in range(NCORES):
        sl = slice(c * SH, (c + 1) * SH)
        vt = np.zeros((3, NF, NAs), np.float32)
        vt[:, :, :SH] = v0[sl].transpose(1, 2, 0)
        st = np.zeros((NF, NAs), np.float32)
        st[:, :SH] = s0[sl].T
        bt = np.zeros((NAs, B), np.float16)
        bt[:SH] = bm[:, sl].T
        in_maps.append({"vT": vt, "sT": st, "bT": bt, **weights})

    key = "prog"
    if key not in _prog_cache:
        _prog_cache[key] = _build_program()
    nc = _prog_cache[key]

    res = run_bass_kernel_spmd(nc, in_maps, list(range(NCORES)))
    if res.exec_time_ns is not None:
        print(f"HW exec time: {res.exec_time_ns} ns")
    kernel._last_result = res

    ysum = np.zeros((B,), np.float64)
    for c in range(NCORES):
        ysum += res.results[c]["y"].reshape(B).astype(np.float64)
    return ysum.astype(np.float32).reshape(B, 1)
